# revision 44
# baseline (speedup 1.0000x reference)
"""BPR-loss Trainium2 kernel.

Loss (see reference): for each graph b with tokens (logits lg, labels lb in
0..3) the per-graph log-prob is the mean over valid soft-labels s in {1,2,3}
of mean_{p: lb=s, n: lb<s} logsigmoid(lg_p - lg_n); loss = -mean over valid
graphs.

Everything after the pairwise logsigmoid is linear, so we:
  1. (host) sort each graph's tokens by label -> "neg" candidates for level s
     become a prefix [0, P_s) and "pos" candidates a contiguous range.
  2. (device, bf16 throughout)  The exp is hoisted to the host via
     exp(neg - pos) = exp(neg) * exp(-pos): the staged inputs are
     exp-space (posrow = exp(-pos) bf16, negcol = exp(neg) f32), so the
     device needs a single Ln activation table:
       - DMA the packed exp(-pos) row [1, Wtot] partition-broadcast to a
         [128, Wtot] tile (bf16: half the DMA bytes of fp32),
       - per stripe (slot x 128-neg-chunk) one in-place DVE
         tensor_scalar_mul against the per-partition exp(neg) column
         (bf16 in/out + f32 scalar hits the DVE 4x perf mode),
       - one wide Ln ACT pass per ~1536-col window:
         ln(1 + u) = softplus(neg-pos) = -logsigmoid(pos-neg),
       - ONE matmul per 512-col super-tile: the stripes' 0/1 prefix-mask
         columns are row-stacked in the lhsT ([128, 3m]) so each stripe
         owns a 3-row block of the psum output at partition strata
         {0,32,64}; every matmul is start=stop=True (chunk partials are
         summed on the host), so 4-bank psum groups [96, 2048] fill up
         and are staged out with one DVE copy + one DMA each.
  3. (host) weight the [3, w] stripe windows by exact pos-masks/scales and
     reduce to the scalar.

8 NeuronCores, data-parallel over graphs: graphs are sorted by work and
dealt in groups of 8 (one per core) so the SPMD program (shapes = group max)
is identical across cores and inherently load-balanced.
"""

import os
import sys

import numpy as np
import ml_dtypes

BF16 = ml_dtypes.bfloat16

for _p in ("/opt/trn_rl_repo", "/root/.axon_site/_ro/trn_rl_repo"):
    if os.path.isdir(_p) and _p not in sys.path:
        sys.path.append(_p)

NCORES = 8
MAXLEN = 256
NLAB = 4  # soft-label count (labels 0..3)
W_SUPER = 512  # super-tile width = one psum bank of f32
W_ACT = 3072  # DVE-multiply / Ln-ACT window (whole supers)
PSUM_W = 512
ALIGN = 2
M_CAP = 10  # stripes per super (3m <= 30 <= 32-row stratum)
KSPLIT = int(os.environ.get("BPR_KSPLIT", "48"))  # K <= KSPLIT -> per-stripe DVE multiply
N_STRATA = 3  # psum partition strata at offsets {0,32,64} (base 96 illegal)
BANKS_PER_GROUP = 4  # psum group = [96, 2048] = 3 strata x 4 banks
SUPERS_PER_GROUP = N_STRATA * BANKS_PER_GROUP
GRPW = BANKS_PER_GROUP * PSUM_W
# number of partition-broadcast input DMAs (one per ACT window if None)
KSORT = os.environ.get("BPR_KSORT", "1") == "1"
KCROP = os.environ.get("BPR_KCROP", "0") == "1"
GRADUATED = os.environ.get("BPR_GRAD", "0") == "1"


def _plan(logits, labels, s_num):
    """Host prep: per-graph label-sort + slotting + packing. Pure numpy."""
    B = int(s_num.shape[0])
    T = int(logits.shape[0])
    s_num = s_num.astype(np.int64)
    ends = np.cumsum(s_num)
    offs = ends - s_num

    graphs = []
    for b in range(B):
        s_eff = int(min(s_num[b], MAXLEN))
        lo = int(min(offs[b], T))
        hi = int(min(lo + s_eff, T))
        lg = logits[lo:hi].astype(np.float32)
        lb = labels[lo:hi].astype(np.int64)
        s_eff = lg.shape[0]
        order = np.argsort(lb, kind="stable")
        lgs = lg[order]
        c = np.bincount(lb, minlength=NLAB)[:NLAB]
        P = np.cumsum(c)  # P[s-1] = #{lb < s}
        P1, P2, P3 = int(P[0]), int(P[1]), int(P[2])
        c0 = int(c[0])
        Cp = s_eff - c0  # pos-candidate count (labels >= 1)
        valid = np.array(
            [(c[s] > 0) and (P[s - 1] > 0) for s in (1, 2, 3)], dtype=bool
        )
        cnt = int(valid.sum())
        gvalid = (int(s_num[b]) > 1) and (cnt > 0)
        # first pos-col whose label's neg-prefix extends past row 128 (the
        # only columns the second n-chunk can contribute to)
        start2 = None
        if P3 > 128:
            for s in (1, 2, 3):
                if P[s - 1] > 128:
                    start2 = sum(int(c[s_]) for s_ in range(1, s))
                    break
        # A[s-1, j]: weight of OUT[s-1, j] (j indexes pos cols = sorted labels
        # 1..3). Nonzero only on the row matching the col's label.
        A = np.zeros((3, max(Cp, 1)), dtype=np.float64)
        if gvalid:
            for s in (1, 2, 3):
                if valid[s - 1]:
                    j0 = int(P[s - 1]) - c0
                    j1 = j0 + int(c[s])
                    A[s - 1, j0:j1] = 1.0 / (float(c[s]) * float(P[s - 1]) * cnt)
        graphs.append(
            dict(
                b=b,
                s_eff=s_eff,
                c0=c0,
                P3=P3,
                Cp=Cp,
                lgs=lgs,
                P=(P1, P2, P3),
                A=A,
                gvalid=gvalid,
                start2=start2,
            )
        )

    n_valid = max(sum(g["gvalid"] for g in graphs), 1)

    # --- slotting: sort by work key desc, deal groups of NCORES ---
    nslots = (B + NCORES - 1) // NCORES
    work = np.array(
        [
            (0 if (g["P3"] == 0 or g["Cp"] == 0) else np.ceil(g["P3"] / 128) * g["Cp"])
            for g in graphs
        ]
    )
    order = np.argsort(-work, kind="stable")
    slots = []  # per slot: members (graph idx per core, may be None), shapes
    for k in range(nslots):
        members = [None] * NCORES
        p3m, cpm = 0, 0
        x2 = None
        for c_ in range(NCORES):
            i = k * NCORES + c_
            if i < B:
                g = graphs[int(order[i])]
                members[c_] = int(order[i])
                if g["P3"] > 0 and g["Cp"] > 0:
                    p3m = max(p3m, g["P3"])
                    cpm = max(cpm, g["Cp"])
                    if g["start2"] is not None:
                        x2 = g["start2"] if x2 is None else min(x2, g["start2"])
        cpm = -(-cpm // ALIGN) * ALIGN
        if p3m == 0 or cpm == 0:
            continue
        if p3m > 128 and x2 is None:
            x2 = 0
        if x2 is not None:
            x2 = (x2 // ALIGN) * ALIGN
        slots.append(dict(members=members, P3=p3m, Cp=cpm, x2=x2))

    # --- stripes: (slot, n-chunk). The second n-chunk only covers the
    # pos-column suffix [x2, Cp) it can contribute to. ---
    stripes = []
    for si, sl in enumerate(slots):
        nch = -(-sl["P3"] // 128)
        for j in range(nch):
            K = min(128, sl["P3"] - 128 * j)
            x0 = 0 if j == 0 else sl["x2"]
            stripes.append(
                dict(slot=si, chunk=j, nch=nch, K=K, W=sl["Cp"] - x0, x0=x0)
            )
    nstripes = len(stripes)

    # Hybrid multiply: big-K stripes get a host-staged exp(neg) broadcast
    # tile (DMA bytes, one tensor_tensor per window); small-K stripes get a
    # per-stripe DVE tensor_scalar against an exp(neg) column (no negb
    # bytes).  Sorting by K desc also lets each window's DMAs and compute
    # crop to the window's max K.
    if KSORT:
        stripes.sort(key=lambda s: -s["K"])
    for st in stripes:
        st["mode"] = "negb" if st["K"] > KSPLIT else "ts"
    n_ts = sum(st["mode"] == "ts" for st in stripes)

    # --- super-tile packing: <=512 cols, <=M_CAP stripes, single mode ---
    supertiles = []
    cur = None
    goff = 0
    for st in stripes:
        if (
            cur is None
            or cur["W"] + st["W"] > W_SUPER
            or len(cur["stripes"]) >= M_CAP
            or cur["mode"] != st["mode"]
        ):
            cur = dict(W=0, g0=goff, stripes=[], mode=st["mode"], Kmax=0)
            supertiles.append(cur)
        st["t"] = len(supertiles) - 1
        st["soff"] = cur["W"]
        st["goff"] = goff
        st["i_local"] = len(cur["stripes"])
        cur["stripes"].append(st)
        cur["W"] += st["W"]
        cur["Kmax"] = max(cur["Kmax"], st["K"])
        goff += st["W"]
    Wtot = goff
    nsupers = len(supertiles)
    ngroups = -(-nsupers // SUPERS_PER_GROUP)

    # --- windows (DVE/ACT granularity): whole supers, single mode each.
    # Graduated: small first window so the pipeline fills early. ---
    if GRADUATED:
        first = min(1024, Wtot)
        last = min(768, max(Wtot - first, 0))
        mid_total = max(Wtot - first - last, 0)
        nmid = max(-(-mid_total // W_ACT), 1)
        budgets = [first] + [-(-mid_total // nmid) + 512] * nmid + [last + 512]
    else:
        budgets = []

    windows = []
    cur = None
    for u, sup in enumerate(supertiles):
        if (
            cur is None
            or (cur["c1"] - cur["c0"]) + sup["W"] > cur["budget"]
            or cur["mode"] != sup["mode"]
        ):
            b = budgets[len(windows)] if len(windows) < len(budgets) else W_ACT
            cur = dict(c0=sup["g0"], c1=sup["g0"], budget=b,
                       mode=sup["mode"], Kmax=0, supers=[])
            windows.append(cur)
        sup["win"] = len(windows) - 1
        cur["c1"] += sup["W"]
        cur["Kmax"] = max(cur["Kmax"], sup["Kmax"])
        cur["supers"].append(u)
    for win in windows:
        win["Kmax"] = (-(-win["Kmax"] // 32) * 32) if KCROP else 128

    # negb region = columns of negb-mode stripes (a prefix after the sort)
    Cneg = max(
        [st["goff"] + st["W"] for st in stripes if st["mode"] == "negb"],
        default=0,
    )

    # super -> (group, stratum, bank): bank-major so low banks finish
    # early and their psum can be staged out while compute continues
    for u, sup in enumerate(supertiles):
        r = u % SUPERS_PER_GROUP
        sup["grp"] = u // SUPERS_PER_GROUP
        sup["stratum"] = r % N_STRATA
        sup["bank"] = r // N_STRATA

    # --- per-core input arrays (exp-space) ---
    posrow = np.zeros((NCORES, max(Wtot, 2)), dtype=BF16)
    negb = np.zeros((NCORES, 128, max(Cneg, 2)), dtype=BF16)
    negcol = np.zeros((NCORES, 128, max(n_ts, 1)), dtype=np.float32)
    bmask = np.zeros((NCORES, 128, max(3 * nstripes, 1)), dtype=BF16)
    jt = 0
    for j, st in enumerate(stripes):
        st["j"] = j  # bmask column block index (emission order)
        if st["mode"] == "ts":
            st["jt"] = jt  # negcol column index
            jt += 1
        sl = slots[st["slot"]]
        for c_ in range(NCORES):
            gi = sl["members"][c_]
            if gi is None:
                continue
            g = graphs[gi]
            if g["P3"] == 0 or g["Cp"] == 0:
                continue
            lgs = g["lgs"]
            c0 = g["c0"]
            x0 = st["x0"]
            if g["Cp"] > x0:
                posrow[c_, st["goff"] : st["goff"] + g["Cp"] - x0] = np.exp(
                    -lgs[c0 + x0 :].astype(np.float64)
                ).astype(BF16)
            n0 = 128 * st["chunk"]
            n1 = min(g["P3"], n0 + 128)
            if n1 > n0:
                en = np.exp(lgs[n0:n1].astype(np.float64))
                if st["mode"] == "negb":
                    negb[
                        c_, 0 : n1 - n0, st["goff"] : st["goff"] + st["W"]
                    ] = en.astype(BF16)[:, None]
                else:
                    negcol[c_, 0 : n1 - n0, st["jt"]] = en.astype(np.float32)
                for s in (1, 2, 3):
                    Ps = g["P"][s - 1]
                    r1 = min(Ps, n1) - n0
                    if r1 > 0:
                        bmask[c_, 0:r1, 3 * j + (s - 1)] = 1.0

    return dict(
        graphs=graphs,
        slots=slots,
        stripes=stripes,
        supertiles=supertiles,
        windows=windows,
        nstripes=nstripes,
        n_ts=n_ts,
        nsupers=nsupers,
        ngroups=ngroups,
        Wtot=Wtot,
        Cneg=Cneg,
        n_valid=n_valid,
        posrow=posrow,
        negb=negb,
        negcol=negcol,
        bmask=bmask,
    )


def _signature(plan):
    """Program-shape signature for caching the compiled module."""
    sig = [plan["Wtot"], plan["nstripes"], plan["nsupers"], plan["Cneg"],
           plan["n_ts"]]
    for st in plan["stripes"]:
        sig += [st["t"], st["K"], st["W"], st["soff"], st["i_local"],
                st["mode"] == "ts"]
    for w in plan["windows"]:
        sig += [w["c0"], w["c1"], w["Kmax"], w["mode"] == "ts"]
    return tuple(sig)


def _out_loc(plan, st):
    """(row0, col0) of a stripe's [3, W] block in the dram out tensor."""
    sup = plan["supertiles"][st["t"]]
    row = 96 * sup["grp"] + 32 * sup["stratum"] + 3 * st["i_local"]
    col = PSUM_W * sup["bank"] + st["soff"]
    return row, col


def _emulate(plan):
    """Numpy emulation of the device program (for correctness of packing)."""
    outs = []
    for c_ in range(NCORES):
        out = np.zeros((plan["ngroups"] * 96, GRPW), dtype=np.float32)
        for st in plan["stripes"]:
            j, w = st["j"], st["W"]
            pos = plan["posrow"][c_][st["goff"] : st["goff"] + w].astype(
                np.float32
            )  # exp(-pos)
            if st["mode"] == "negb":
                neg = plan["negb"][c_][
                    0 : st["K"], st["goff"] : st["goff"] + w
                ].astype(np.float32)  # exp(neg) broadcast
            else:
                neg = plan["negcol"][c_][0 : st["K"], st["jt"]][:, None]
            u = (pos[None, :] * neg).astype(BF16).astype(np.float32)
            val = np.log1p(u).astype(BF16).astype(np.float32)
            bm = plan["bmask"][c_][0 : st["K"], 3 * j : 3 * j + 3]
            acc = bm.astype(np.float32).T @ val  # [3, w]
            r0, c0 = _out_loc(plan, st)
            out[r0 : r0 + 3, c0 : c0 + w] = acc
        outs.append(out.astype(BF16).astype(np.float32))
    return outs


def _epilogue(plan, outs):
    total = 0.0
    for c_ in range(NCORES):
        out = np.asarray(outs[c_]).astype(np.float32)
        for j, st in enumerate(plan["stripes"]):
            sl = plan["slots"][st["slot"]]
            gi = sl["members"][c_]
            if gi is None:
                continue
            g = plan["graphs"][gi]
            if g["P3"] == 0 or g["Cp"] == 0 or not g["gvalid"]:
                continue
            x0 = st["x0"]
            w_eff = g["Cp"] - x0
            if w_eff <= 0:
                continue
            r0, c0 = _out_loc(plan, st)
            O = out[r0 : r0 + 3, c0 : c0 + w_eff].astype(np.float64)
            total += float((g["A"][:, x0 : x0 + w_eff] * O).sum())
    # device computes softplus(-(pos-neg)) = -logsigmoid(pos-neg); the loss
    # is -mean(logsigmoid) so the signs cancel.
    return np.float32(total / plan["n_valid"])


_PROG_CACHE = {}


def _build_program(plan):
    import concourse.bass as bass  # noqa: F401
    import concourse.tile as tile
    from concourse import bacc, mybir
    from contextlib import ExitStack

    f32 = mybir.dt.float32
    bf16 = mybir.dt.bfloat16
    nc = bacc.Bacc("TRN2", target_bir_lowering=False, debug=False,
                   num_devices=NCORES)
    Wtot = max(plan["Wtot"], 2)
    nstripes = max(plan["nstripes"], 1)
    posrow = nc.dram_tensor("posrow", [1, Wtot], bf16, kind="ExternalInput")
    negbd = nc.dram_tensor("negb", [128, max(plan["Cneg"], 2)], bf16,
                           kind="ExternalInput")
    negcd = nc.dram_tensor("negcol", [128, max(plan["n_ts"], 1)], f32,
                           kind="ExternalInput")
    bmask = nc.dram_tensor("bmask", [128, 3 * nstripes], bf16,
                           kind="ExternalInput")
    out = nc.dram_tensor("out", [max(plan["ngroups"], 1) * 96, GRPW], bf16,
                         kind="ExternalOutput")

    LN = mybir.ActivationFunctionType.Ln

    stripes, sups = plan["stripes"], plan["supertiles"]
    windows = plan["windows"]

    # readback chunks: (grp, bank-pair).  Bank-major super order means a
    # chunk's 6 supers are consecutive, so its psum region is final (and
    # can be staged out) while later supers still compute.
    chunks = {}  # (grp, pair) -> dict(last super, col range)
    for u, sup in enumerate(sups):
        key = (sup["grp"], sup["bank"] // 2)
        ch = chunks.setdefault(
            key, dict(last=u, cb0=sup["bank"] * PSUM_W, cb1=0)
        )
        ch["last"] = u
        ch["cb0"] = min(ch["cb0"], sup["bank"] * PSUM_W)
        ch["cb1"] = max(ch["cb1"], sup["bank"] * PSUM_W + PSUM_W)
    last_chunk_of_super = {ch["last"]: (key, ch) for key, ch in chunks.items()}

    with tile.TileContext(nc) as tc, ExitStack() as ctx:
        cpool = ctx.enter_context(tc.tile_pool(name="const", bufs=1))
        dpool = ctx.enter_context(
            tc.tile_pool(name="d", bufs=len(windows) or 1)
        )
        npool = ctx.enter_context(
            tc.tile_pool(name="n", bufs=len(windows) or 1)
        )
        vpool = ctx.enter_context(
            tc.tile_pool(name="v", bufs=len(windows) or 1)
        )
        ppool = ctx.enter_context(tc.tile_pool(name="ps", bufs=2, space="PSUM"))
        spool = ctx.enter_context(tc.tile_pool(name="stage", bufs=8))

        # input DMAs: bt first (every matmul waits on it and DMA completion
        # latency is ~5us), then the windows in order, each posb (and a
        # negb-window's negb) row-cropped to the window's Kmax — rows
        # beyond it are never touched by compute.
        bt = cpool.tile([128, 3 * nstripes], bf16, tag="bt")
        nc.sync.dma_start(out=bt[:, :], in_=bmask.ap()[:, :])
        dts, nbs = {}, {}
        negt = None
        for w, win in enumerate(windows):
            ww = win["c1"] - win["c0"]
            km = win["Kmax"]
            dts[w] = dpool.tile([128, ww], bf16, name=f"d{w}", tag="d")
            nc.sync.dma_start(
                out=dts[w][0:km, :],
                in_=posrow.ap()[0:1, win["c0"] : win["c1"]]
                .partition_broadcast(km),
            )
            if win["mode"] == "negb":
                nbs[w] = npool.tile([128, ww], bf16, name=f"n{w}", tag="n")
                nc.sync.dma_start(
                    out=nbs[w][0:km, :],
                    in_=negbd.ap()[0:km, win["c0"] : win["c1"]],
                )
            elif negt is None:
                negt = cpool.tile([128, max(plan["n_ts"], 1)], f32,
                                  tag="negt")
                nc.sync.dma_start(out=negt[:, :], in_=negcd.ap()[:, :])

        ptiles = {}
        for grp in range(plan["ngroups"]):
            ptiles[grp] = ppool.tile([96, GRPW], f32, tag="ps",
                                     name=f"ps{grp}")

        ncopy = 0
        for w, win in enumerate(windows):
            dt_ = dts[w]
            wc0 = win["c0"]
            km = win["Kmax"]
            # u = exp(-pos) * exp(neg): one tensor_tensor per negb window,
            # or one tensor_scalar per stripe in a ts window
            if win["mode"] == "negb":
                nc.vector.tensor_tensor(
                    dt_[0:km, :], dt_[0:km, :], nbs[w][0:km, :],
                    mybir.AluOpType.mult,
                )
            else:
                for u in win["supers"]:
                    for st in sups[u]["stripes"]:
                        go, sw = st["goff"] - wc0, st["W"]
                        nc.vector.tensor_scalar_mul(
                            dt_[0 : st["K"], go : go + sw],
                            dt_[0 : st["K"], go : go + sw],
                            negt[0 : st["K"], st["jt"] : st["jt"] + 1],
                        )
            # one Ln pass per window: ln(1 + u) = softplus(neg-pos)
            vt_ = vpool.tile([128, win["c1"] - wc0], bf16, name=f"v{w}",
                             tag="v")
            nc.scalar.activation(
                vt_[0:km, :], dt_[0:km, :], LN, bias=1.0, scale=1.0
            )
            # one matmul per super: stripes row-stacked via the mask block
            for u in win["supers"]:
                sup = sups[u]
                slist = sup["stripes"]
                m = len(slist)
                j0 = slist[0]["j"]
                pt = ptiles[sup["grp"]]
                r0 = 32 * sup["stratum"]
                cb = PSUM_W * sup["bank"]
                nc.tensor.matmul(
                    out=pt[r0 : r0 + 3 * m, cb : cb + sup["W"]],
                    lhsT=bt[0:km, 3 * j0 : 3 * j0 + 3 * m],
                    rhs=vt_[0:km, sup["g0"] - wc0 : sup["g0"] - wc0 + sup["W"]],
                    start=True,
                    stop=True,
                )
                if u in last_chunk_of_super:
                    (grp, pair), ch = last_chunk_of_super[u]
                    cw = ch["cb1"] - ch["cb0"]
                    stg = spool.tile([96, cw], bf16, tag="stage",
                                     name=f"stg{grp}_{pair}")
                    if ncopy % 2 == 0:
                        nc.scalar.copy(stg[:, :], pt[:, ch["cb0"] : ch["cb1"]])
                    else:
                        nc.vector.tensor_copy(
                            stg[:, :], pt[:, ch["cb0"] : ch["cb1"]]
                        )
                    ncopy += 1
                    nc.sync.dma_start(
                        out=out.ap()[
                            96 * grp : 96 * grp + 96, ch["cb0"] : ch["cb1"]
                        ],
                        in_=stg[:, :],
                    )
    nc.compile()
    return nc


def _run_device(plan, trace=False):
    from concourse.bass_utils import run_bass_kernel_spmd

    sig = _signature(plan)
    if sig not in _PROG_CACHE:
        _PROG_CACHE[sig] = _build_program(plan)
    nc = _PROG_CACHE[sig]
    in_maps = [
        {
            "posrow": plan["posrow"][c_][None, :],
            "negb": np.ascontiguousarray(plan["negb"][c_]),
            "negcol": np.ascontiguousarray(plan["negcol"][c_]),
            "bmask": np.ascontiguousarray(plan["bmask"][c_]),
        }
        for c_ in range(NCORES)
    ]
    res = run_bass_kernel_spmd(
        nc, in_maps, core_ids=list(range(NCORES)), trace=trace
    )
    kernel._last_results = res
    return [res.results[c_]["out"] for c_ in range(NCORES)]


def kernel(logits, labels, s_num, _emulate_only=False, _trace=False):
    logits = np.asarray(logits)
    labels = np.asarray(labels)
    s_num = np.asarray(s_num)
    plan = _plan(logits, labels, s_num)
    if plan["nstripes"] == 0:
        return np.float32(0.0)
    if _emulate_only:
        outs = _emulate(plan)
    else:
        outs = _run_device(plan, trace=_trace)
    return _epilogue(plan, outs)


kernel._last_results = None
